# revision 1
# baseline (speedup 1.0000x reference)
"""Trainium2 Bass kernel for nn_BlockSelfAttentionModule.

Math: out[b*H+h, l, m] = sum_d q[b*H+h, l, d] * R_h[l, m, d]
  R_h[l, m, :] = r_voice[l%8, m%8, :, h]
               + (e_past[fi-fj, :, h] if fj <= fi else e_future[fj-fi, :, h])
  with fi = l//8, fj = m//8.

Key property: along a row l, the 384 output columns take only 8 (voice) + 48
(time) distinct values:  out[l, m] = V[l, m%8] + T[l, m//8].

Per core (head h, all 4 batches b, all 3 row-chunks c of 128):
  Z_b = qT_b.T @ W48          (one 128x477 matmul per b; W48 is block-diagonal
                               so all 3 chunks' Z come out side by side)
    per-chunk W cols: G[v] = e_future[47-v] (v<47) | e_past[v-47] (v in [47,95))
                      U[95+di*8+do] = r_voice[di,do]
  G-blocks are copied from PSUM into a gather buffer at column
  S(b,c) = 384b + 112c, so the time gather for row l = 128c + 8fi' + di is
    ts[l, j*48+k] = gsb[l, 128j + fi' + k]      (j = 3b+c)
  which needs only ONE op per fi' (16 total): the per-partition-group shift
  fi' is constant inside each op, and the (b,c) loop merges into a single
  stride-128 dim.  The 16 gathers are spread over ACT/DVE copies (fp with a
  32-aligned partition base) and both HWDGE rings + SWDGE DMA pipelines.
  T[l, fj] = ts[l, j*48+47-fj].
  The voice part stages U columns to SBUF, then selects
    vt[l, 12r + 3b + c] = usb[l, b*192 + c*64 + 8*(l%8) + r]
  with 8 copy_predicated ops (one per di residue, masked by a host-supplied
  p%8 mask), batched over all 4 batches via 4-D APs.
  Final: out[l, fj*8+r] = ts[l, j*48+47-fj] + vt[l, j*8+r] -- tensor_adds
  per (b,c) on DVE/Pool using stride-0 / negative-stride read APs, then one
  merged 590 KB store per batch (the last one split across both rings).

Notable hardware constraints baked into the AP choices:
  - DMA APs: max 3 dims, contiguous final dim; an SBUF dim stride must be a
    pure partition-stride (multiple of the row length) or stay in-row; a
    partition-comb (stride > 1 partition) dim only lowers correctly from
    offset 0.
  - Compute-engine APs must start at partition 0/32/64/96.
  - fp32 PE matmul is quarter rate; float32r is full rate but rounds inputs
    (~2e-4 rel err, rejected here for accuracy), and needs an even N.

Sharding: head-parallel, core h handles head h (4 batch rows of the output).
"""

import os
import sys

for _p in ("/opt/trn_rl_repo", "/root/.axon_site/_ro/trn_rl_repo"):
    if os.path.isdir(_p) and _p not in sys.path:
        sys.path.insert(0, _p)

import contextlib

import numpy as np

import concourse.bass as bass
import concourse.bacc as bacc
import concourse.mybir as mybir
import concourse.tile as tile
from concourse import masks
from concourse.bass_utils import run_bass_kernel_spmd

E, H, DI, DO, F = 16, 8, 8, 8, 48
L = F * DI  # 384
B = 4  # batch
NGR = 95  # G columns per chunk (time part)
NW = NGR + DI * DO  # 159 W columns per chunk
NZ = 3 * NW  # 477: block-diagonal W48 gives all 3 l-chunks in one matmul
NZP = NZ + 1  # 478: fp32r matmul requires an even moving-dim size
NJ = 3 * B  # 12 (b,c) pairs
RG = 128 * NJ  # 1536: gsb row length (G region only)
NCORES = 8
DT = mybir.dt.float32

_prog_cache = {}

# mask[p, di] = 1.0 where p % 8 == di (for the voice copy_predicated select)
MSK = np.ascontiguousarray(
    (np.arange(128)[:, None] % 8 == np.arange(8)[None, :]).astype(np.uint8)
)


def build_program(loop_n=None):
    nc = bacc.Bacc("TRN2", target_bir_lowering=False, debug=False)
    qh = nc.dram_tensor("qh", [B, L, E], DT, kind="ExternalInput")
    wh = nc.dram_tensor("wh", [48, NZP], DT, kind="ExternalInput")
    mk = nc.dram_tensor("mk", [128, 8], mybir.dt.uint8, kind="ExternalInput")
    out = nc.dram_tensor("out", [B, L, L], DT, kind="ExternalOutput")

    with tile.TileContext(nc) as tc, contextlib.ExitStack() as ctx:
        const_pool = ctx.enter_context(tc.tile_pool(name="const", bufs=1))
        qts_pool = ctx.enter_context(tc.tile_pool(name="qts", bufs=2))
        qtp_pool = ctx.enter_context(tc.tile_pool(name="qtp", bufs=1, space="PSUM"))
        zp_pool = ctx.enter_context(tc.tile_pool(name="zp", bufs=4, space="PSUM"))
        osb_pool = ctx.enter_context(tc.tile_pool(name="osb", bufs=4))

        loop_ctx = tc.For_i(0, loop_n, 1) if loop_n else contextlib.nullcontext()
        ctx.enter_context(loop_ctx)

        ident = const_pool.tile([128, 128], DT)
        masks.make_identity(nc, ident[:])
        W = const_pool.tile([48, NZP], DT)
        # two gather buffers, one per batch pair: wave-A gathers (b0/b1)
        # overlap the b2/b3 matmuls, and b0/b1 adds overlap wave B
        gsb_a = const_pool.tile([128, RG // 2], DT)
        gsb_b = const_pool.tile([128, RG // 2], DT)
        ts12 = const_pool.tile([128, NJ * F], DT)
        vt12 = const_pool.tile([128, NJ * 8], DT)
        msk = const_pool.tile([128, 8], mybir.dt.uint8)
        nc.sync.dma_start(msk[:], mk[:])
        # U columns staged to SBUF so the voice selects batch over all b
        usb = const_pool.tile([128, B * 192], DT)

        # q for all batches in one DMA: (128, 192); col j*16 + d holds
        # q[b, c*128 + p, d] on partition p, j = 3b + c.  Issued first: it
        # gates the whole PE chain.
        qsb = const_pool.tile([128, B * 48], DT)
        q_src0 = bass.AP(qh, 0, [[E, 128], [128 * E, 6], [1, E]])
        q_dst0 = bass.AP(qsb.tensor, 0, [[B * 48, 128], [16, 6], [1, 16]])
        nc.sync.dma_start(q_dst0, q_src0)
        nc.scalar.dma_start(W[:], wh[:])
        q_src1 = bass.AP(qh, 6 * 128 * E, [[E, 128], [128 * E, 6], [1, E]])
        q_dst1 = bass.AP(qsb.tensor, 96, [[B * 48, 128], [16, 6], [1, 16]])
        nc.scalar.dma_start(q_dst1, q_src1)

        # all transposes first, then matmuls back-to-back: keeps the PE busy
        # continuously so it ramps to full clock
        qts_tiles = []
        for b in range(B):
            qt_psum = qtp_pool.tile([48, 128], DT, tag=f"qtp{b}")
            nc.tensor.transpose(qt_psum[:], qsb[:, b * 48:(b + 1) * 48], ident[:])
            qts = qts_pool.tile([48, 128], DT, tag=f"qts{b}", name=f"qts{b}")
            nc.vector.tensor_copy(qts[:], qt_psum[:])
            qts_tiles.append(qts)

        for b in range(B):
            # Z for all 3 chunks at once: psum (128, 478)
            z_psum = zp_pool.tile([128, NZP], DT, tag="zp")
            nc.tensor.matmul(z_psum[:], qts_tiles[b][:], W[:])

            # place all 3 G blocks with one strided copy (dst col 384b'+112c+v)
            gsb = gsb_a if b < 2 else gsb_b
            g_dst = bass.AP(
                gsb.tensor, 384 * (b % 2), [[RG // 2, 128], [112, 3], [1, NGR]]
            )
            g_src = bass.AP(z_psum.tensor, 0, [[NZP, 128], [NW, 3], [1, NGR]])
            nc.scalar.copy(g_dst, g_src)

            # stage U cols to SBUF: usb[l, b*192 + c*64 + u] = Z[l, c*159+95+u]
            u_dst = bass.AP(usb.tensor, b * 192, [[B * 192, 128], [64, 3], [1, 64]])
            u_src = bass.AP(z_psum.tensor, NGR, [[NZP, 128], [NW, 3], [1, 64]])
            nc.vector.tensor_copy(u_dst, u_src)

        # voice select from SBUF, one op per (di, batch-pair) so the early
        # pair's selects finish before the late matmuls:
        # vt12[l, 12r + 3b + c] = usb[l, b*192 + c*64 + 8*(l%8) + r]
        for half in (0, 1):
            for di in range(8):
                data = bass.AP(
                    usb.tensor, half * 384 + 8 * di,
                    [[B * 192, 128], [192, 2], [1, 8], [64, 3]],
                )
                mask = bass.AP(msk.tensor, di, [[8, 128], [0, 2], [0, 8], [0, 3]])
                vout = bass.AP(
                    vt12.tensor, half * 6,
                    [[NJ * 8, 128], [3, 2], [NJ, 8], [1, 3]],
                )
                nc.vector.copy_predicated(vout, mask, data)

        # time skew gather: one DMA per fi' (8 consecutive partitions)
        # ts12[8*fi'+di, j*48+k] = gsb[8*fi'+di, 128j + fi' + k]
        # per-fp the shift is constant, so compute engines can gather too;
        # spread across ACT/DVE copies, both HWDGE rings and SWDGE.  Wave A
        # (j 0..5, from gsb_a) only needs b0/b1; wave B follows b2/b3.
        NJH = NJ // 2
        for wave, g in ((0, gsb_a), (1, gsb_b)):
            for fp in range(16):
                src = bass.AP(
                    g.tensor, 8 * fp * (RG // 2) + fp,
                    [[RG // 2, 8], [128, NJH], [1, F]],
                )
                dst = bass.AP(
                    ts12.tensor, 8 * fp * (NJ * F) + wave * NJH * F,
                    [[NJ * F, 8], [F, NJH], [1, F]],
                )
                if fp == 0 or fp == 8:
                    nc.scalar.copy(dst, src)  # compute quads: base 0/64
                elif fp == 4 or fp == 12:
                    nc.vector.tensor_copy(dst, src)  # base 32/96
                elif fp in (1, 5, 9, 13, 2):
                    nc.sync.dma_start(dst, src)
                elif fp in (3, 7, 11, 15, 6):
                    nc.scalar.dma_start(dst, src)
                else:
                    nc.gpsimd.dma_start(dst, src)

        # final broadcast-add per (b, c); one merged store per b
        for b in range(B):
            osb = osb_pool.tile([128, 3 * L], DT, tag="osb")
            for c in range(3):
                j = 3 * b + c
                t_b = bass.AP(ts12.tensor, j * F + 47, [[NJ * F, 128], [-1, F], [0, 8]])
                v_b = bass.AP(vt12.tensor, j, [[NJ * 8, 128], [0, F], [NJ, 8]])
                o_ap = bass.AP(osb.tensor, c * L, [[3 * L, 128], [8, F], [1, 8]])
                eng = nc.gpsimd if c == 0 else nc.vector
                eng.tensor_add(o_ap, t_b, v_b)
            # out[b, c*128 + p, m] <- osb[p, c*384 + m]; the last batch's
            # store is split across both HWDGE rings to shorten the tail
            if b < 3:
                st_src = bass.AP(osb.tensor, 0, [[3 * L, 128], [L, 3], [1, L]])
                st_dst = bass.AP(out, b * L * L, [[L, 128], [128 * L, 3], [1, L]])
                st_eng = [nc.sync, nc.scalar, nc.sync][b]
                st_eng.dma_start(st_dst, st_src)
            else:
                for half, eng in ((0, nc.sync), (1, nc.scalar)):
                    st_src = bass.AP(
                        osb.tensor, half * 64 * 3 * L,
                        [[3 * L, 64], [L, 3], [1, L]],
                    )
                    st_dst = bass.AP(
                        out, b * L * L + half * 64 * L,
                        [[L, 64], [128 * L, 3], [1, L]],
                    )
                    eng.dma_start(st_dst, st_src)

    nc.compile()
    return nc


def _get_program():
    if "nc" not in _prog_cache:
        _prog_cache["nc"] = build_program()
    return _prog_cache["nc"]


def make_core_inputs(q, r_voice, e_past, e_future):
    """Host-side sharding: per-head q slice + per-head parameter matrix W48."""
    q = np.ascontiguousarray(q, dtype=np.float32)
    in_maps = []
    for h in range(NCORES):
        qh = np.ascontiguousarray(q.reshape(B, H, L, E)[:, h])  # (4, 384, 16)
        W = np.empty((E, NW), dtype=np.float32)
        # G cols (unreversed): col v = e_future[47-v] for v<47, e_past[v-47] after
        W[:, 0:47] = e_future[1:48, :, h][::-1].T
        W[:, 47:95] = e_past[:, :, h].T
        W[:, NGR:] = r_voice[:, :, :, h].reshape(DI * DO, E).T
        W48 = np.zeros((48, NZP), dtype=np.float32)
        for c in range(3):
            W48[c * 16:(c + 1) * 16, c * NW:(c + 1) * NW] = W
        in_maps.append({"qh": qh, "wh": np.ascontiguousarray(W48), "mk": MSK})
    return in_maps


def kernel(q, flipped_masks, r_voice, e_past, e_future):
    q = np.asarray(q, dtype=np.float32)
    r_voice = np.asarray(r_voice, dtype=np.float32)
    e_past = np.asarray(e_past, dtype=np.float32)
    e_future = np.asarray(e_future, dtype=np.float32)

    nc = _get_program()
    in_maps = make_core_inputs(q, r_voice, e_past, e_future)
    res = run_bass_kernel_spmd(nc, in_maps, core_ids=list(range(NCORES)))

    out = np.empty((B * H, L, L), dtype=np.float32)
    for h in range(NCORES):
        out_h = res.results[h]["out"]  # (4, 384, 384)
        for b in range(B):
            out[b * H + h] = out_h[b]
    return out



# revision 13
# speedup vs baseline: 1.4168x; 1.4168x over previous
"""Trainium2 Bass kernel for nn_BlockSelfAttentionModule (v3).

Math: out[b*H+h, l, m] = sum_d q[b*H+h, l, d] * R_h[l, m, d]
  R_h[l, m, :] = r_voice[l%8, m%8, :, h]
               + (e_past[fi-fj, :, h] if fj <= fi else e_future[fj-fi, :, h])
  with fi = l//8, fj = m//8.  out[l, m] = T[l, m//8] + V[l, m%8].

Layout (per core = head h): partition p = 32*u + 4*di + b; l-tiles t = 0..11
cover frames fi = 4t + u, l = 32t + 8u + di.  Design points:
 - q arrives HOST-PRETRANSPOSED as the matmul lhsT and is CONCATENATED with
   the weight matrix into one dram tensor -> a single load DMA gates all
   four matmuls (two serialized transfers + sem hops otherwise).
 - 4 float32r matmuls (full PE rate at N>=256; v1 used quarter-rate fp32),
   N = 348 = 3 tau-blocks of [51 G | 64 U | 1 pad], one PSUM bank per g.
 - the skew shift u is constant per 32-partition QUAD, so the time gather is
   16 plain 3-D copies at legal partition bases 0/32/64/96 on DVE/ACT
   reading PSUM directly (v1: ~20 tiny DMAs through the shared HWDGE ring;
   GPSIMD cannot touch PSUM, so Pool gets only SBUF work).
 - voice path: U columns staged PSUM->SBUF, then copy_predicated per
   (di, g-pair) with host mask mk[p, di] = ((p//4)%8 == di).
 - output is bf16 (tolerance 2e-2 >> bf16 ~4e-3): halves store bytes and
   enables the DVE 2x 16-bit mode for += ops with packed last dims.
 - final add osb[p, tau*384+fj*8+r] = ts[p, tau*48+47-fj] + vt[p, ...r] per
   tile: ACT/Pool broadcast-expand of ts + DVE packed-2x "+= vt", or a Pool
   fused add (ACT cannot tensor_add).
 - work tensors are SPLIT per consumer scope (ts per g, usb/vt per g-pair,
   osb per store) so cross-iteration WAR dependencies in the For_i loop
   retire per-tile instead of tail-to-head: multi-buf tiles would lose
   subtile dependency tracking entirely (any >=2-buf or 4-D-AP access is
   treated as a whole-tensor write, serializing every accessor pair).
 - out DRAM is PARTITION-MAJOR [t, p, m] so stores are fully contiguous:
   4 DMAs of 3 tiles each.  Host un-permutes with one numpy transpose.

Sharding: head-parallel, core h handles head h (4 batch rows of the output).
"""

import os
import sys

for _p in ("/opt/trn_rl_repo", "/root/.axon_site/_ro/trn_rl_repo"):
    if os.path.isdir(_p) and _p not in sys.path:
        sys.path.insert(0, _p)

import contextlib

import numpy as np

import concourse.bass as bass
import concourse.bacc as bacc
import concourse.mybir as mybir
import concourse.tile as tile
from concourse.bass_utils import run_bass_kernel_spmd

E, H, DI, DO, F = 16, 8, 8, 8, 48
L = F * DI  # 384
B = 4
NG = 51             # G window cols per tau-block
NU = DI * DO        # 64 voice cols
NBLK = NG + NU + 1  # 116 padded block width
NMM = 3 * NBLK      # 348: matmul N (even, >=256 for fp32r full rate)
GSTR = 512          # PSUM bank stride
NW = 4 * NMM        # 1392 W48 cols
NQW = 512 + NW      # 1904: fused [qt | W48] row length
NCORES = 8
DT = mybir.dt.float32
DTR = mybir.dt.float32r
DTH = mybir.dt.bfloat16

_prog_cache = {}

# mk[p, di] = 1 where (p//4) % 8 == di (voice copy_predicated select)
MSK = np.ascontiguousarray(
    ((np.arange(128)[:, None] // 4) % 8 == np.arange(8)[None, :]).astype(np.uint8)
)


def build_program(loop_n=None):
    nc = bacc.Bacc("TRN2", target_bir_lowering=False, debug=False)
    qw = nc.dram_tensor("qw", [48, NQW], DTR, kind="ExternalInput")
    mk = nc.dram_tensor("mk", [128, 8], mybir.dt.uint8, kind="ExternalInput")
    out = nc.dram_tensor("out", [12, 128, L], DTH, kind="ExternalOutput")

    with tile.TileContext(nc) as tc, contextlib.ExitStack() as ctx:
        const_pool = ctx.enter_context(tc.tile_pool(name="const", bufs=1))
        q_pool = ctx.enter_context(tc.tile_pool(name="q", bufs=2))
        zp_pool = ctx.enter_context(tc.tile_pool(name="zp", bufs=1, space="PSUM"))
        wk_pool = ctx.enter_context(tc.tile_pool(name="wk", bufs=1))

        loop_ctx = tc.For_i(0, loop_n, 1) if loop_n else contextlib.nullcontext()
        ctx.enter_context(loop_ctx)

        qws = q_pool.tile([48, NQW], DTR, tag="qws")
        nc.sync.dma_start(qws[:], qw[:])
        msk = const_pool.tile([128, 8], mybir.dt.uint8)
        nc.sync.dma_start(msk[:], mk[:])

        z_tiles = []
        for g in range(4):
            z_g = zp_pool.tile([128, GSTR], DT, tag=f"z{g}")
            nc.tensor.matmul(
                z_g[:, 0:NMM],
                qws[:, g * 128:(g + 1) * 128],
                qws[:, 512 + g * NMM:512 + (g + 1) * NMM],
            )
            z_tiles.append(z_g)

        # per-scope work tiles: fine-grained cross-iteration WAR retirement
        usb = [wk_pool.tile([128, 384], DTH, tag=f"usb{gp}") for gp in range(2)]
        tsg = [wk_pool.tile([128, 144], DTH, tag=f"ts{g}") for g in range(4)]
        vtg = [wk_pool.tile([128, 48], DTH, tag=f"vt{gp}") for gp in range(2)]
        osb = [wk_pool.tile([128, 3 * L], DTH, tag=f"osb{j}") for j in range(4)]

        def u_copy(g):
            # usb[gp][p, (g%2)*192 + tau*64 + 8di + do]  (ACT, from PSUM)
            u_src = bass.AP(
                z_tiles[g].tensor, NG, [[GSTR, 128], [NBLK, 3], [1, NU]]
            )
            u_dst = bass.AP(
                usb[g // 2].tensor, (g % 2) * 192, [[384, 128], [NU, 3], [1, NU]]
            )
            nc.scalar.copy(u_dst, u_src)

        def gather(u, g):
            # ts[g][p, tau*48 + k] = Z[p, tau*116 + u + k], 3-D from PSUM
            src = bass.AP(
                z_tiles[g].tensor, (32 * u) * GSTR + u,
                [[GSTR, 32], [NBLK, 3], [1, F]],
            )
            dst = bass.AP(
                tsg[g].tensor, (32 * u) * 144, [[144, 32], [F, 3], [1, F]]
            )
            if u % 2 == 1:
                nc.scalar.copy(dst, src)
            else:
                nc.vector.tensor_copy(dst, src)

        def voice(di, gp):
            # vt[gp][p, (g%2)*24 + tau*8 + r], predicated on di(p) == di (DVE)
            data = bass.AP(
                usb[gp].tensor, 8 * di, [[384, 128], [192, 2], [1, 8], [NU, 3]]
            )
            mask = bass.AP(msk.tensor, di, [[8, 128], [0, 2], [0, 8], [0, 3]])
            vout = bass.AP(vtg[gp].tensor, 0, [[48, 128], [24, 2], [1, 8], [8, 3]])
            nc.vector.copy_predicated(vout, mask, data)

        def t_aps(g, tau):
            t_ap = bass.AP(
                tsg[g].tensor, tau * F + 47, [[144, 128], [-1, F], [0, 8]]
            )
            v_ap = bass.AP(
                vtg[g // 2].tensor, (g % 2) * 24 + tau * 8,
                [[48, 128], [0, F], [1, 8]],
            )
            o_ap = bass.AP(
                osb[g].tensor, tau * L, [[3 * L, 128], [8, F], [1, 8]]
            )
            return t_ap, v_ap, o_ap

        def expand(g, tau, eng):
            t_ap, _, o_ap = t_aps(g, tau)
            if eng is nc.scalar:
                nc.scalar.copy(o_ap, t_ap)
            else:
                nc.gpsimd.tensor_copy(o_ap, t_ap)

        def pass2(g, tau):
            _, v_ap, o_ap = t_aps(g, tau)
            nc.vector.tensor_add(o_ap, o_ap, v_ap)  # packed bf16 2x +=

        def fused(g, tau):
            t_ap, v_ap, o_ap = t_aps(g, tau)
            nc.gpsimd.tensor_add(o_ap, t_ap, v_ap)

        def store(j):
            st_src = bass.AP(osb[j].tensor, 0, [[3 * L, 128], [L, 3], [1, L]])
            st_dst = bass.AP(
                out, j * 3 * 128 * L, [[L, 128], [128 * L, 3], [1, L]]
            )
            nc.sync.dma_start(st_dst, st_src)

        # per triple (g): tau0 expand-pair (ACT if g<2 else Pool) + DVE 2x,
        # tau1 Pool fused, tau2 expand-pair (Pool if g<2 else ACT) + DVE 2x
        u_copy(0), u_copy(1)
        for g in range(2):
            gather(0, g); gather(2, g)     # DVE
            gather(1, g); gather(3, g)     # ACT
        for di in range(8):
            voice(di, 0)                   # DVE
        u_copy(2), u_copy(3)
        for g in range(2, 4):
            gather(0, g); gather(2, g)
            gather(1, g); gather(3, g)
        for di in range(8):
            voice(di, 1)
        for g in range(4):
            expand(g, 0, nc.scalar if g < 2 else nc.gpsimd)
            expand(g, 2, nc.gpsimd if g < 2 else nc.scalar)
            fused(g, 1)                    # Pool
            pass2(g, 0)
            pass2(g, 2)
            store(g)

    nc.compile()
    return nc


def _get_program():
    if "nc" not in _prog_cache:
        _prog_cache["nc"] = build_program()
    return _prog_cache["nc"]


def make_core_inputs(q, r_voice, e_past, e_future):
    """Host-side sharding: per-head [pretransposed-q | block-diag W48]."""
    q = np.ascontiguousarray(q, dtype=np.float32)
    qr = q.reshape(B, H, L, E)
    in_maps = []
    for h in range(NCORES):
        qh = qr[:, h]  # (B, L, E)
        # lhsT[16*tau + d, g*128 + p] = q[b, l, d],
        # p = 32u + 4di + b, l = 8*(12g + 4tau + u) + di
        lt = qh.reshape(B, 4, 3, 4, 8, E)      # (b, g, tau, u, di, d)
        lt = lt.transpose(2, 5, 1, 3, 4, 0)    # (tau, d, g, u, di, b)
        qw = np.zeros((48, NQW), dtype=np.float32)
        qw[:, :512] = lt.reshape(48, 4 * 128)
        master = np.zeros((E, 95), dtype=np.float32)
        master[:, :47] = e_future[1:48, :, h][::-1].T
        master[:, 47:] = e_past[:, :, h].T
        U = r_voice[:, :, :, h].reshape(DI * DO, E).T
        for g in range(4):
            for tau in range(3):
                c0 = 512 + g * NMM + tau * NBLK
                F0 = 4 * (3 * g + tau)
                qw[16 * tau:16 * tau + 16, c0:c0 + NG] = master[:, F0:F0 + NG]
                qw[16 * tau:16 * tau + 16, c0 + NG:c0 + NG + NU] = U
        in_maps.append({"qw": np.ascontiguousarray(qw), "mk": MSK})
    return in_maps


def kernel(q, flipped_masks, r_voice, e_past, e_future):
    q = np.asarray(q, dtype=np.float32)
    r_voice = np.asarray(r_voice, dtype=np.float32)
    e_past = np.asarray(e_past, dtype=np.float32)
    e_future = np.asarray(e_future, dtype=np.float32)

    nc = _get_program()
    in_maps = make_core_inputs(q, r_voice, e_past, e_future)
    res = run_bass_kernel_spmd(nc, in_maps, core_ids=list(range(NCORES)))

    # device out is [t, p, m] with p = 32u + 4di + b; un-permute to l-major
    # (l = 32t + 8u + di) and cast bf16 -> f32 host-side.
    out = np.empty((B * H, L, L), dtype=np.float32)
    for h in range(NCORES):
        arr = np.asarray(res.results[h]["out"], dtype=np.float32)
        arr = arr.reshape(12, 4, 8, B, L).transpose(3, 0, 1, 2, 4)  # (b,t,u,di,m)
        for b in range(B):
            out[b * H + h] = arr[b].reshape(L, L)
    return out
